# revision 72
# baseline (speedup 1.0000x reference)
"""BiPhaseScorer Trainium2 kernel (8 NeuronCores, SPMD). v7

Sharding: core (b, g) = batch b in {0,1} x head-group g in {0..3} (2 heads each).
Each core: projects its batch's tokens onto its 2 heads' QKV slices, runs
bi-phase attention (phase + magnitude scores, softmax), applies its heads'
slice of the output projections, and writes per-core partial outputs
[S, E] for x and y channels (bf16). Host sums partials over head-groups,
adds biases. 148us (v2 baseline) -> 127.6us in the cost model.

Math: cos(arctan2(y,x)) = x/r, sin = y/r with r = sqrt(x^2+y^2), so
  scores = C1*(cosq.cosk + sinq.sink) + C2*(rq.rk),  C1=BETA/D, C2=(1-B)/sqrt(D)
The whole score contraction is ONE fp8 DoubleRow matmul per chunk: per head
a [128, 2, S] fp8e4m3 tensor with k-tile 0 = [cos;sin] and k-tile 1
holding sqrt(8)*r. K-side k-tile 1 is the natural [r_h0; r_h1] packing
(identical for both heads - one lane-local stt + a Pool copy, no DMA);
only the Q side needs its off-head half zeroed (memset once) to kill the
cross-head leak, since zeros on one side of the product suffice.
DoubleRow contracts both k-tiles at 0.5 cycles/row: 256 PE cycles/chunk
vs 1024 for the v2 f32r pair. sqrt(8)^2 * C1 == C2, so the matmul yields
scores/C1 and the softmax exp applies scale=C1. fp8 quantization of
cos/sin/r adds ~4e-3 on top of v2's 7.6e-3 (fp8 Q/K input streams).
NOTE: es / V in fp8 measured 4e-2 (FAIL). The attention here is near
uniform (~2000 effective keys), so the output is itself a 1/sqrt(2000)-
scale average and per-weight quantization noise does NOT shrink relative
to it - attn weights and V must stay bf16.

DMA-queue discipline (the HWDGE queue charges ~625ns fixed per DMA, and
all transfers serialize on a shared engine pool): Q/K activations load in
512/1024-col blocks front-loaded so ALL K/Q projection units run in the
first ~18us, V quarters deferred behind them (attnV has po-serial slack),
the 6 projection weights in 3 paired tensors ([E, 2, D2]: 512B descriptor
runs instead of 256B which would eat a 2x small-element penalty), the 4
QK biases in one [4*D2] tensor (loaded FIRST - they gate the first
Square), out-projections in bf16, and outputs staged as [128, 2, E] bf16
pairs (16 output DMAs instead of 32, half the bytes). Only the sin
partition shifts (2 per side-tile) remain as small DMAs.

Emission order IS the Tile scheduler's priority order, and each engine
executes its queue near-priority-order, so the program is emitted as an
explicit software-pipelined schedule: score+exp+denominator units (su)
for ALL qts weave through the phase-A projection units as soon as their
kd8/qd8 tiles can exist (decoupled from attnV via deep es buffering);
attnV units (au) follow qt-serially (po lives in PSUM); each qt's finish
(fh) and out-proj overlap the next qt's streams. su lookahead is capped
at ~1 qt: further ahead, the es buffer rotation would cycle through the
po/finish chain and deadlock.

Phase B unit detail: a [128, 2, 512] PSUM supertile takes two DoubleRow
score matmuls and ONE exp covers both (halves ACT per-op access
overhead). Denominator: per-super pair add + running sum (bf16 2x adds
on DVE; scalar_tensor_tensor would NOT get a perf mode - two tensor
reads use both ports; Pool is ~3.7x slower per element and cannot read
PSUM at all), finished by a gpsimd partition_all_reduce which also
broadcasts. Projections drain out of PSUM immediately (DVE, bias add
folded into the drain) so the 2 psA rotation banks free fast; out-proj
staging splits DVE (x) / ACT (y). sqrt via ln+exp keeps ACT on one
act-table set (redundant LoadActFuncSet insts merged post-finalize).

Layouts (per core):
  qd8[h][qt]/kd8[h][kt] [128, 2, 512] fp8e4m3 (see above)
  v_sb            [128, 256] bf16 per key-chunk: free = per-head 128 cols,
                  h0 = [vx|vy], h1 = [vy|vx] (swap lets O^T land lane-local
                  in the x/y-stacked ot tensors below)
  otx             [128, S] bf16: [h0 Ox^T (0:64); h1 Ox^T (64:128)]
  oty             [128, S] bf16: [h1 Oy^T (0:64); h0 Oy^T (64:128)]
Out-proj is then a single Kc=128 matmul per (token block, channel).
"""

import numpy as np
import ml_dtypes

import concourse.bacc as bacc
import concourse.mybir as mybir
from concourse.tile import TileContext
from concourse import bass_isa
from concourse.bass_utils import run_bass_kernel_spmd

B, S, E, H = 2, 2048, 512, 8
D = E // H              # 64
BETA = 0.5
SCALE = float(np.sqrt(D))
C1 = BETA / D                       # exp() input scale
C2 = (1.0 - BETA) / SCALE
R8 = float(np.sqrt(C2 / C1))        # sqrt(8): per-side r scale

NCORES = 8
HG = 2
D2 = HG * D             # 128
EC = E // 128           # 4
TT = 4                  # 512-token tiles
QT = 4
KC = S // 128           # 16

F32 = mybir.dt.float32
F32R = mybir.dt.float32r
BF16 = mybir.dt.bfloat16
F8 = mybir.dt.float8e4

TRACE = False
LAST_RESULTS = None

ADD = mybir.AluOpType.add
MULT = mybir.AluOpType.mult
AF = mybir.ActivationFunctionType
DR = mybir.MatmulPerfMode.DoubleRow


def build_bass(stage="full"):
    nc = bacc.Bacc("TRN2", target_bir_lowering=False, debug=False,
                   enable_asserts=True, num_devices=NCORES)

    xs = {}
    for n in ["xqx", "xqy", "xkx", "xky"]:
        xs[n] = nc.dram_tensor(n, [E, S], F8, kind="ExternalInput")
    for n in ["xvx", "xvy"]:
        xs[n] = nc.dram_tensor(n, [E, S], BF16, kind="ExternalInput")
    # paired projection weights: [:, 0, :] = x-channel, [:, 1, :] = y-channel
    ws = {n: nc.dram_tensor(n, [E, 2, D2], BF16, kind="ExternalInput")
          for n in ["wk", "wq", "wv"]}
    # x/y-stacked output projections [128, E] (see module docstring)
    wox = nc.dram_tensor("wox", [D2, E], BF16, kind="ExternalInput")
    woy = nc.dram_tensor("woy", [D2, E], BF16, kind="ExternalInput")
    # [bqx, bqy, bkx, bky] concatenated
    ball = nc.dram_tensor("ball", [4 * D2], F32, kind="ExternalInput")
    yx = nc.dram_tensor("yx", [S, E], BF16, kind="ExternalOutput")
    yy = nc.dram_tensor("yy", [S, E], BF16, kind="ExternalOutput")

    with TileContext(nc) as tc:
        with (
            tc.tile_pool(name="persist", bufs=1) as pp,
            tc.tile_pool(name="wpool", bufs=1) as wp,
            tc.tile_pool(name="stream", bufs=4) as sp,
            tc.tile_pool(name="tmp", bufs=3) as tp,
            tc.tile_pool(name="psA", bufs=1, space="PSUM") as psA,
            tc.tile_pool(name="psB", bufs=1, space="PSUM") as psB,
        ):
            qd8 = [[pp.tile([128, 2, 512], F8, tag=f"qd8_{h}_{t}",
                            name=f"qd8_{h}_{t}") for t in range(QT)]
                   for h in range(HG)]
            kd8 = [[pp.tile([128, 2, 512], F8, tag=f"kd8_{h}_{t}",
                            name=f"kd8_{h}_{t}") for t in range(TT)]
                   for h in range(HG)]
            v_sb = [pp.tile([128, 2 * D2], BF16, tag=f"v_sb{t}", name=f"v_sb{t}")
                    for t in range(KC)]
            otx = [pp.tile([128, 512], BF16, tag=f"otx{t}", name=f"otx{t}") for t in range(QT)]
            oty = [pp.tile([128, 512], BF16, tag=f"oty{t}", name=f"oty{t}") for t in range(QT)]

            # zero the off-head r half of the Q-side k-tile-1 once (zeros on
            # one side of the product kill the cross-head leak; the K side
            # keeps the natural full [r_h0; r_h1] packing)
            for h in range(HG):
                zsl = slice(64, 128) if h == 0 else slice(0, 64)
                for t in range(QT):
                    nc.gpsimd.memset(qd8[h][t][zsl, 1, :], 0.0)

            w_sb = {n: wp.tile([128, EC, 2, D2], BF16, tag=f"w_{n}",
                               name=f"w_{n}") for n in ws}
            wox_sb = wp.tile([D2, E], BF16, tag="wox")
            woy_sb = wp.tile([D2, E], BF16, tag="woy")
            b_all = wp.tile([D2, 4], F32, tag="ball")
            bx_of = {"wq": b_all[:, 0:1], "wk": b_all[:, 2:3]}
            by_of = {"wq": b_all[:, 1:2], "wk": b_all[:, 3:4]}

            def load_w(n):
                nc.sync.dma_start(w_sb[n][:], ws[n].ap().rearrange(
                    "(c p) i d -> p c i d", p=128))

            eps_sb = wp.tile([128, 1], F32, tag="eps")
            nc.vector.memset(eps_sb[:], 1e-20)

            def load_qk(name, lo, hi, chunks=1):
                """Load columns [lo*512, hi*512) of a Q/K tensor; returns
                (tile, lo) so users can rebase column offsets. chunks>1
                splits the transfer so the first columns land earlier."""
                w = (hi - lo) * 512
                xt = sp.tile([128, EC, w], F8, tag=f"xt_f8_{hi - lo}",
                             bufs=(4 if hi - lo == 1 else 6),
                             name=f"{name}_{lo}")
                cw = w // chunks
                for c in range(chunks):
                    nc.sync.dma_start(
                        xt[:, :, c * cw:(c + 1) * cw],
                        xs[name].ap().rearrange("(c p) t -> p c t", p=128)
                        [:, :, lo * 512 + c * cw:lo * 512 + (c + 1) * cw])
                return (xt, lo)

            def load_v(name, q):
                tsl = slice(q * 512, (q + 1) * 512)
                xt = sp.tile([128, EC, 512], BF16, tag="xt_bf16", bufs=4,
                             name=f"{name}_{q}")
                nc.sync.dma_start(xt[:], xs[name].ap().rearrange(
                    "(c p) t -> p c t", p=128)[:, :, tsl])
                return xt

            def qk_side(xa_p, xb_p, nw, d8, tt, split=1):
                """xa_p/xb_p are (tile, base_colblock) pairs; process 512
                columns at tt*512. split=2 pipelines in 256-column halves
                so the first half of the fp8 tensors is usable earlier."""
                xa, alo = xa_p
                xb, blo = xb_p
                assert alo == blo
                pa_ps = psA.tile([128, 512], F32, tag="proj", bufs=2, name="pa")
                pb = psA.tile([128, 512], F32, tag="proj", bufs=2, name="pb")
                co = (tt - alo) * 512
                W = 512 // split
                for hf in range(split):
                    sl = slice(hf * W, (hf + 1) * W)
                    xsl = slice(co + hf * W, co + (hf + 1) * W)
                    for ec in range(EC):
                        nc.tensor.matmul(pa_ps[:, sl], w_sb[nw][:, ec, 0],
                                         xa[:, ec, xsl],
                                         start=(ec == 0), stop=(ec == EC - 1))
                    # drain both projections to SBUF immediately, with the
                    # bias add folded into the drain (DVE - Pool cannot read
                    # PSUM): the psA rotation buffers free right after one
                    # copy, so the next unit's matmuls pipeline instead of
                    # waiting out this unit's whole ACT/DVE chain
                    va = tp.tile([128, W], F32, tag="pacp", bufs=3, name="va")
                    nc.vector.tensor_scalar_add(va[:], pa_ps[:, sl], bx_of[nw])
                    for ec in range(EC):
                        nc.tensor.matmul(pb[:, sl], w_sb[nw][:, ec, 1],
                                         xb[:, ec, xsl],
                                         start=(ec == 0), stop=(ec == EC - 1))
                    vb = tp.tile([128, W], F32, tag="pbcp", bufs=3, name="vb")
                    nc.vector.tensor_scalar_add(vb[:], pb[:, sl], by_of[nw])
                    t0 = tp.tile([128, W], F32, tag="t0", bufs=2, name="t0")
                    t1 = tp.tile([128, W], F32, tag="t1", bufs=2, name="t1")
                    nc.scalar.activation(t0[:], va[:], AF.Square)
                    nc.scalar.activation(t1[:], vb[:], AF.Square)
                    nc.vector.tensor_add(t0[:], t0[:], t1[:])
                    # r = sqrt(t0) via ln/exp: ACT stays on one table set
                    lu = tp.tile([128, W], F32, tag="lu", bufs=2, name="lu")
                    nc.scalar.activation(lu[:], t0[:], AF.Ln,
                                         bias=eps_sb[:])
                    rc = tp.tile([128, W], F32, tag="rc", bufs=2, name="rc")
                    nc.scalar.activation(rc[:], lu[:], AF.Exp, scale=-0.5)
                    with nc.allow_low_precision(reason="fp8 score operands"):
                        sn = tp.tile([128, W], F8, tag="sn", bufs=4, name="sn")
                        nc.vector.tensor_mul(sn[:], vb[:], rc[:])
                        # r*sqrt(8) = (t0*sqrt(8))*rc since rc = 1/sqrt(t0)
                        if d8 is kd8:
                            # K side: both heads share the natural [r0; r1]
                            # packing - write h0's tensor, Pool-copy to h1's
                            nc.vector.scalar_tensor_tensor(
                                d8[0][tt][:, 1, sl], t0[:], float(R8), rc[:],
                                op0=MULT, op1=MULT)
                            nc.gpsimd.tensor_copy(d8[1][tt][:, 1, sl],
                                                  d8[0][tt][:, 1, sl])
                        else:
                            # Q side: lane-local halves (other half is zero)
                            nc.vector.scalar_tensor_tensor(
                                d8[0][tt][0:64, 1, sl], t0[0:64], float(R8),
                                rc[0:64], op0=MULT, op1=MULT)
                            nc.vector.scalar_tensor_tensor(
                                d8[1][tt][64:128, 1, sl], t0[64:128], float(R8),
                                rc[64:128], op0=MULT, op1=MULT)
                        # cos halves straight into the per-head k-tile-0
                        # stacks (lane-local)
                        nc.vector.tensor_mul(d8[0][tt][0:64, 0, sl],
                                             va[0:64], rc[0:64])
                        nc.vector.tensor_mul(d8[1][tt][64:128, 0, sl],
                                             va[64:128], rc[64:128])
                    # sin needs a partition shift (the only per-tile DMAs)
                    nc.sync.dma_start(d8[0][tt][64:128, 0, sl], sn[0:64])
                    nc.sync.dma_start(d8[1][tt][0:64, 0, sl], sn[64:128])

            def v_sub(tb):
                """Project V key-chunk tb (128 tokens) into v_sb[tb]."""
                xvx_t, xvy_t = xv[tb // 4]
                ssl = slice((tb % 4) * 128, (tb % 4 + 1) * 128)
                pv = psA.tile([128, 512], F32, tag="proj", bufs=2,
                              name="pv")[:, 0:2 * D2]
                for ec in range(EC):
                    nc.tensor.matmul(pv[:, 0:D2], xvx_t[:, ec, ssl],
                                     w_sb["wv"][:, ec, 0],
                                     start=(ec == 0), stop=False)
                for ec in range(EC):
                    nc.tensor.matmul(pv[:, D2:2 * D2], xvy_t[:, ec, ssl],
                                     w_sb["wv"][:, ec, 1],
                                     start=(ec == 0), stop=(ec == EC - 1))
                # psum input-major [vx_h0|vx_h1|vy_h0|vy_h1] ->
                # v_sb h0 = [vx_h0|vy_h0], h1 = [vy_h1|vx_h1]
                vt = v_sb[tb][:].rearrange("p (i z) -> p i z", i=4)
                pvv = pv[:].rearrange("p (i z) -> p i z", i=4)
                with nc.allow_low_precision(reason="bf16 V"):
                    nc.vector.tensor_copy(vt[:, 0::2], pvv[:, 0::3])
                    nc.vector.tensor_copy(vt[:, 1::2], pvv[:, 2:0:-1])

            # DMA issue order follows first use; the tiny ball (biases) goes
            # first - it gates the first Square, and the first K tiles load
            # as 512-col quarters so the first projections start ~2us in.
            # Later loads are emitted LATER (inside the block schedule):
            # emission order is the scheduler's priority, and bulk transfers
            # issued up front would hog the serial DMA devices ahead of the
            # small sin-shift DMAs that complete qd8/kd8 and gate phase B.
            load_w("wk")
            xk = {0: (load_qk("xkx", 0, 1), load_qk("xky", 0, 1))}
            nc.sync.dma_start(b_all[:], ball.ap().rearrange("(i p) -> p i", p=128))
            load_w("wq")
            xq01 = (load_qk("xqx", 0, 2), load_qk("xqy", 0, 2))
            xq = {0: xq01, 1: xq01}
            xk[1] = (load_qk("xkx", 1, 2), load_qk("xky", 1, 2))
            load_w("wv")
            xv = {}

            def ku(tt, split=1):
                qk_side(xk[tt][0], xk[tt][1], "wk", kd8, tt, split=split)

            def qu(tt):
                qk_side(xq[tt][0], xq[tt][1], "wq", qd8, tt)

            def out_proj(qt):
                # oy staging rides Pool mid-kernel (ACT is the bottleneck);
                # the last qt keeps it on ACT so the tail's x/y copies run on
                # two engines in parallel
                for half in range(2):
                    oxp = tp.tile([128, 2, E], BF16, tag="oxp", bufs=2, name="oxp")
                    oyp = tp.tile([128, 2, E], BF16, tag="oyp", bufs=2, name="oyp")
                    for i in range(2):
                        sub = half * 2 + i
                        sb_ = sub * 128
                        pyx = psA.tile([128, E], F32, tag="proj", bufs=2,
                                       name="pyx")
                        pyy = psA.tile([128, E], F32, tag="proj", bufs=2,
                                       name="pyy")
                        nc.tensor.matmul(pyx[:], otx[qt][:, sb_:sb_ + 128],
                                         wox_sb[:], start=True, stop=True)
                        nc.tensor.matmul(pyy[:], oty[qt][:, sb_:sb_ + 128],
                                         woy_sb[:], start=True, stop=True)
                        with nc.allow_low_precision(reason="bf16 partials"):
                            # Pool cannot read PSUM: stage on DVE (x) and
                            # ACT (y) so the two channels copy in parallel
                            nc.vector.tensor_copy(oxp[:, i], pyx[:])
                            nc.scalar.activation(oyp[:, i], pyy[:],
                                                 AF.Identity)
                    tb = qt * 4 + half * 2
                    tsl = slice(tb * 128, (tb + 2) * 128)
                    nc.sync.dma_start(
                        yx.ap()[tsl, :].rearrange("(s p) e -> p s e", p=128),
                        oxp[:])
                    nc.sync.dma_start(
                        yy.ap()[tsl, :].rearrange("(s p) e -> p s e", p=128),
                        oyp[:])

            # phase B blocks (see module docstring). Emission order IS the
            # per-engine execution order, so phase-B units are interleaved
            # with the phase-A units that feed them: qt0's blocks weave
            # between K/Q/V projection units, and each qt's first blocks
            # (which need no po buffer / no attnV yet) are emitted before the
            # previous qt's finish chain so ACT never drains at qt
            # boundaries.
            KS = KC // 2
            st = {}

            def su(qt, ks):
                """Scores + exp + denominator partials for super-chunk ks of
                qt (both heads). Decoupled from attnV: exps run as early as
                their kd8/qd8 allow and park in es buffers."""
                if ks == 0:
                    st[qt] = dict(po=None, es_l=[[], []], quads=[[], []])
                s = st[qt]
                for h in range(HG):
                    kc0 = 2 * ks
                    kt, ko = kc0 // 4, (kc0 % 4) * 128
                    ps = psB.tile([128, 2, 512], F32, tag="s", bufs=2,
                                  name="ps")
                    nc.tensor.matmul(ps[:, 0], kd8[h][kt][:, :, ko:ko + 128],
                                     qd8[h][qt][:, :, :],
                                     start=True, stop=True, perf_mode=DR)
                    nc.tensor.matmul(ps[:, 1],
                                     kd8[h][kt][:, :, ko + 128:ko + 256],
                                     qd8[h][qt][:, :, :],
                                     start=True, stop=True, perf_mode=DR)
                    es = tp.tile([128, 2, 512], BF16, tag="es", bufs=26,
                                 name="es")
                    with nc.allow_low_precision(reason="bf16 attn weights"):
                        nc.scalar.activation(es[:], ps[:], AF.Exp,
                                             scale=float(C1))
                    s["es_l"][h].append(es)
                    with nc.allow_low_precision(reason="bf16 denominator"):
                        p = tp.tile([128, 512], BF16, tag="pair",
                                    bufs=8, name="pair")
                        nc.vector.tensor_add(p[:], es[:, 0], es[:, 1])
                        if not s["quads"][h]:
                            s["quads"][h].append(p)
                        else:
                            r2 = tp.tile([128, 512], BF16, tag="run",
                                         bufs=8, name="run")
                            nc.vector.tensor_add(r2[:], s["quads"][h][-1][:],
                                                 p[:])
                            s["quads"][h].append(r2)

            def au(qt, ks):
                """attnV + denominator for super-chunk ks of qt (both
                heads); po/den accumulation is qt-serial (PSUM-resident).
                Both are single fp8 DoubleRow matmuls over the es supertile;
                the ones-matmul accumulates the full softmax denominator
                (pair + partition + chunk sums) into one PSUM row per head
                (h0 at partition 0, h1 at partition 32 via tile_position)."""
                s = st[qt]
                if ks == 0:
                    s["po"] = [psB.tile([128, 512], F32, tag="o", bufs=2,
                                        name=f"po{h}") for h in range(HG)]
                kc0 = 2 * ks
                for h in range(HG):
                    ep = s["es_l"][h][ks]
                    pv_ = v_sb[kc0][:, h * D2:(h + 1) * D2]
                    nc.tensor.matmul(s["po"][h][:], pv_, ep[:, 0],
                                     start=(ks == 0), stop=False)
                    pv_ = v_sb[kc0 + 1][:, h * D2:(h + 1) * D2]
                    nc.tensor.matmul(s["po"][h][:], pv_, ep[:, 1],
                                     start=False, stop=(ks == KS - 1))

            def fh_den(qt, h):
                """Denominator finish: needs only the su-side run chain, so
                it overlaps the last attnV matmuls of the block."""
                s = st[qt]
                qq = s["quads"][h][-1]
                den_bc = tp.tile([128, 512], F32, tag="denb", bufs=3, name="den_bc")
                nc.gpsimd.partition_all_reduce(den_bc[:], qq[:], channels=128,
                                               reduce_op=bass_isa.ReduceOp.add)
                bc_sb = tp.tile([128, 512], F32, tag="bcs", bufs=3, name="bc_sb")
                with nc.allow_low_precision(reason="softmax denom reciprocal"):
                    nc.vector.reciprocal(bc_sb[:], den_bc[:])
                s.setdefault("bc", {})[h] = bc_sb

            def fh(qt, h):
                s = st[qt]
                po = s["po"]
                bc_sb = s["bc"][h]
                # h0: po = [Ox^T; Oy^T] -> otx[0:64], oty[64:128]
                # h1: po = [Oy^T; Ox^T] -> oty[0:64], otx[64:128]
                lo, hi = (otx[qt], oty[qt]) if h == 0 else (oty[qt], otx[qt])
                with nc.allow_low_precision(reason="bf16 attn out"):
                    if qt == QT - 1:
                        # tail: column-halved muls let the first out-proj
                        # matmuls start one half earlier
                        for cs_ in (slice(0, 256), slice(256, 512)):
                            nc.vector.tensor_mul(lo[0:64, cs_],
                                                 po[h][0:64, cs_],
                                                 bc_sb[0:64, cs_])
                            nc.vector.tensor_mul(hi[64:128, cs_],
                                                 po[h][64:128, cs_],
                                                 bc_sb[64:128, cs_])
                    else:
                        nc.vector.tensor_mul(lo[0:64, :], po[h][0:64],
                                             bc_sb[0:64])
                        nc.vector.tensor_mul(hi[64:128, :], po[h][64:128],
                                             bc_sb[64:128])

            # emission schedule (= scheduler priority): su units emit as
            # soon as their kd8/qd8 halves can exist, across ALL qts, so ACT
            # always has exp work; au units follow qt-serially (po PSUM);
            # V-projection units and deferred loads slot between.
            def loadk23():
                xk23 = (load_qk("xkx", 2, 4), load_qk("xky", 2, 4))
                xk[2] = xk[3] = xk23
            def loadq23():
                xq23 = (load_qk("xqx", 2, 4), load_qk("xqy", 2, 4))
                xq[2] = xq[3] = xq23
            def loadv(q):
                xv[q] = (load_v("xvx", q), load_v("xvy", q))
            def loadwo():
                nc.sync.dma_start(wox_sb[:], wox.ap())
                nc.sync.dma_start(woy_sb[:], woy.ap())

            # K/Q units all run first (their data is front-loaded) so every
            # qt's exp stream unlocks by ~18us; V projections defer (attnV
            # has po-serial slack anyway)
            ku(0); qu(0)
            su(0, 0); su(0, 1)
            ku(1); qu(1)
            loadk23(); loadq23()
            su(0, 2); su(0, 3); su(1, 0); su(1, 1)
            loadv(0); loadv(1)
            su(1, 2); su(1, 3)
            v_sub(0); v_sub(1); au(0, 0)
            v_sub(2); v_sub(3); au(0, 1)
            ku(2); qu(2)
            su(0, 4); su(0, 5); su(1, 4); su(1, 5)
            loadv(2); loadv(3)
            v_sub(4); v_sub(5); au(0, 2)
            v_sub(6); v_sub(7); au(0, 3)
            ku(3); qu(3)
            su(0, 6); su(0, 7)
            loadwo()
            v_sub(8); v_sub(9); au(0, 4)
            su(1, 6); su(1, 7)
            v_sub(10); v_sub(11); au(0, 5)
            su(2, 0); su(2, 1)
            v_sub(12); v_sub(13); au(0, 6)
            v_sub(14); v_sub(15); au(0, 7)
            su(2, 2); su(2, 3)
            fh_den(0, 0); fh_den(0, 1)
            fh(0, 0); fh(0, 1)
            # ~1-qt su lookahead from here: su weaves through the previous
            # qt's au stream - further ahead and the es slot rotation would
            # cycle through the po/finish chain (deadlock)
            au(1, 0); su(2, 4)
            au(1, 1); su(2, 5)
            out_proj(0)
            au(1, 2); su(2, 6)
            au(1, 3); su(2, 7)
            au(1, 4); su(3, 0)
            au(1, 5); su(3, 1)
            au(1, 6); su(3, 2)
            fh_den(1, 0); fh_den(1, 1)
            au(1, 7); su(3, 3)
            fh(1, 0); fh(1, 1)
            au(2, 0); su(3, 4)
            au(2, 1); su(3, 5)
            out_proj(1)
            au(2, 2); su(3, 6)
            au(2, 3); su(3, 7)
            au(2, 4); au(2, 5); au(2, 6)
            fh_den(2, 0); fh_den(2, 1)
            au(2, 7)
            fh(2, 0); fh(2, 1)
            au(3, 0); au(3, 1)
            out_proj(2)
            au(3, 2); au(3, 3); au(3, 4); au(3, 5); au(3, 6)
            fh_den(3, 0); fh_den(3, 1)
            au(3, 7)
            fh(3, 0); fh(3, 1)
            out_proj(3)

    nc.finalize()
    _merge_act_table_loads(nc)
    return nc


def _merge_act_table_loads(nc):
    """All ACT funcs this kernel uses (Square/Ln/Exp) live together in act
    table set 6 (natural_log_exp_and_others), but the insertion pass assigns
    each func its first-matching set (square/exp->0, ln->5) and thrashes
    12+ reloads. Retarget every load to set 6 and drop the now-redundant
    ones (only those carrying no semaphore waits/updates)."""
    for blk in nc.m.functions[0].blocks:
        seen = False
        keep = []
        for inst in blk.instructions:
            if isinstance(inst, mybir.InstLoadActFuncSet):
                si = inst.sync_info
                has_sync = si is not None and (
                    len(si.on_wait) > 0 or len(si.on_update) > 0)
                if seen and not has_sync:
                    continue  # redundant reload, safe to drop
                inst.act_func_set_id = 6
                seen = True
            keep.append(inst)
        blk.instructions[:] = keep


_NC_CACHE = None


def make_in_maps(acts, W, bias):
    """acts: dict qx..vy [B,S,E] f32; W: dict Wqx..Woy; bias: dict bqx..boy."""
    f32 = np.float32
    bf16 = ml_dtypes.bfloat16
    in_maps = []
    for core in range(NCORES):
        b, g = core // 4, core % 4
        gs = slice(g * D2, (g + 1) * D2)
        h0 = slice((2 * g) * D, (2 * g + 1) * D)
        h1 = slice((2 * g + 1) * D, (2 * g + 2) * D)
        m = {}
        f8 = ml_dtypes.float8_e4m3
        m["xqx"] = np.ascontiguousarray(acts["qx"][b].T).astype(f8)
        m["xqy"] = np.ascontiguousarray(acts["qy"][b].T).astype(f8)
        m["xkx"] = np.ascontiguousarray(acts["kx"][b].T).astype(f8)
        m["xky"] = np.ascontiguousarray(acts["ky"][b].T).astype(f8)
        m["xvx"] = np.ascontiguousarray(acts["vx"][b].T).astype(bf16)
        m["xvy"] = np.ascontiguousarray(acts["vy"][b].T).astype(bf16)
        for pn, nx, ny in [("wq", "Wqx", "Wqy"), ("wk", "Wkx", "Wky"),
                           ("wv", "Wvx", "Wvy")]:
            m[pn] = np.ascontiguousarray(np.stack(
                [W[nx][gs].T, W[ny][gs].T], axis=1)).astype(bf16)
        # otx partitions = (h0 dx, h1 dx); oty partitions = (h1 dy, h0 dy)
        m["wox"] = np.ascontiguousarray(np.concatenate(
            [W["Wox"][:, h0].T, W["Wox"][:, h1].T], axis=0)).astype(bf16)
        m["woy"] = np.ascontiguousarray(np.concatenate(
            [W["Woy"][:, h1].T, W["Woy"][:, h0].T], axis=0)).astype(bf16)
        m["ball"] = np.ascontiguousarray(np.concatenate(
            [bias["bqx"][gs], bias["bqy"][gs],
             bias["bkx"][gs], bias["bky"][gs]]))
        in_maps.append(m)
    return in_maps


def kernel(qx, qy, kx, ky, vx, vy,
           Wqx, bqx, Wqy, bqy, Wkx, bkx, Wky, bky,
           Wvx, bvx, Wvy, bvy, Wox, box, Woy, boy):
    global _NC_CACHE, LAST_RESULTS
    f32 = np.float32
    acts = {"qx": qx, "qy": qy, "kx": kx, "ky": ky, "vx": vx, "vy": vy}
    acts = {k: np.asarray(v, f32) for k, v in acts.items()}
    W = {"Wqx": Wqx, "Wqy": Wqy, "Wkx": Wkx, "Wky": Wky,
         "Wvx": Wvx, "Wvy": Wvy, "Wox": Wox, "Woy": Woy}
    W = {k: np.asarray(v, f32) for k, v in W.items()}
    bias = {"bqx": bqx, "bqy": bqy, "bkx": bkx, "bky": bky,
            "bvx": bvx, "bvy": bvy}
    bias = {k: np.asarray(v, f32) for k, v in bias.items()}
    box, boy = np.asarray(box, f32), np.asarray(boy, f32)

    if _NC_CACHE is None:
        _NC_CACHE = build_bass()
    nc = _NC_CACHE

    in_maps = make_in_maps(acts, W, bias)
    # device execution can flake (NRT_EXEC_UNIT_UNRECOVERABLE observed once
    # on an otherwise-identical program) -> retry once before giving up
    try:
        res = run_bass_kernel_spmd(nc, in_maps, core_ids=list(range(NCORES)),
                                   trace=TRACE)
    except Exception:
        import time
        time.sleep(5)
        res = run_bass_kernel_spmd(nc, in_maps, core_ids=list(range(NCORES)),
                                   trace=TRACE)
    LAST_RESULTS = res

    out_x = np.zeros((B, S, E), f32)
    out_y = np.zeros((B, S, E), f32)
    for core in range(NCORES):
        b = core // 4
        out_x[b] += np.asarray(res.results[core]["yx"], f32)
        out_y[b] += np.asarray(res.results[core]["yy"], f32)
    out_x += box + bias["bvx"] @ W["Wox"].T
    out_y += boy + bias["bvy"] @ W["Woy"].T
    return out_x, out_y
